# revision 37
# baseline (speedup 1.0000x reference)
"""Trainium2 Bass kernel for nn_Attention_27358941675773.

Reference computation (per batch b):
    q = x @ Q              [N, H]
    k = x @ K              [N, H]
    V = V_down @ V_up      [L, L]
    v = x @ V              [N, L]
    S = q @ k.T / 256      [N, N]
    out = softmax(S) @ v   [N, L]

Sharding: pure data-parallel over batch B=8 across the 8 NeuronCores
(one batch element per core); small params replicated. No collectives.

Per-core kernel strategy (N=4096, L=256, H=128):
  - Inputs are shipped as fp16 (x transposed to [L, N]); all matmuls run
    at full PE rate (1 cyc/row). qT [H,N] and kT [H,N] are computed
    directly in transposed layout so scores can be built as S_T[m, n]
    (keys on partitions) without any transposes.
  - The value path is factored through the rank-H bottleneck:
        out = softmax(S) @ x @ V_down @ V_up
    so the O(N^2) matmul contracts into H=128 columns (w = x @ V_down),
    and V_up is applied after the softmax-normalization - halving the
    PE work of the attention*value product.
  - exp(S_T/256) runs on the Scalar engine straight out of PSUM, written
    as bf16 (scores reach ~±70; exp stays in fp32/bf16 range, so no
    max-subtraction pass is needed).
  - softmax denominator rowsum[n] = sum_m exp(S_T[m,n]): two levels of
    pairwise adds on the Vector engine, then an 8-chunk ones-vector
    matmul accumulated in PSUM (partition-axis reduction).
  - normalization: 1/rowsum is partition-broadcast on GpSimd and applied
    to mid^T = w^T-weighted numerator with one Vector multiply; the
    output is stored TRANSPOSED [L, N] in DRAM and un-transposed on the
    host during the gather.
  - The attention*w matmul of block k-1 is software-pipelined against
    the QK/exp of block k so the Scalar engine's exp stream stays hidden.
"""

import os
import sys

import numpy as np

for _p in ("/opt/trn_rl_repo",):
    if _p not in sys.path and os.path.isdir(_p):
        sys.path.insert(0, _p)

B, N, L, H = 8, 4096, 256, 128
SCALER = 256.0
NB = 1024           # query-block (free dim of score tiles)
NBH = 512           # half block (one PSUM bank of fp32)
NT = N // NB        # 4 query blocks
MT = N // 128       # 32 key tiles of 128
P = 128


def _build():
    import concourse.bass as bass
    import concourse.tile as tile
    from concourse import bacc, bass_isa, mybir
    from contextlib import ExitStack

    f32 = mybir.dt.float32
    f16 = mybir.dt.float16
    bf16 = mybir.dt.bfloat16
    AF = mybir.ActivationFunctionType

    nc = bacc.Bacc(
        "TRN2", target_bir_lowering=False, debug=False, num_devices=B
    )

    xT_ext = nc.declare_dram_parameter("xT", [L, N], f16, isOutput=False)
    wq_ext = nc.declare_dram_parameter("Wq", [L, H], f16, isOutput=False)
    wk_ext = nc.declare_dram_parameter("Wk", [L, H], f16, isOutput=False)
    vd_ext = nc.declare_dram_parameter("Vd", [L, H], f16, isOutput=False)
    vu_ext = nc.declare_dram_parameter("Vu", [H, L], f16, isOutput=False)
    # output stored transposed [L, N]; host un-transposes at gather
    out_ext = nc.declare_dram_parameter("out", [L, N], f16, isOutput=True)

    with tile.TileContext(nc) as tc, ExitStack() as ctx:
        persist = ctx.enter_context(tc.tile_pool(name="persist", bufs=1))

        ones_bf = persist.tile([P, 1], bf16)
        nc.gpsimd.memset(ones_bf[:], 1.0)
        # touch Exp right away so the ~2.7us ACT table load overlaps the
        # input DMAs instead of delaying the first real exp
        dum = persist.tile([1, 2], f32)
        nc.gpsimd.memset(dum[:], 0.0)
        nc.scalar.activation(dum[:, 1:2], dum[:, 0:1], AF.Exp)

        qw16 = persist.tile([P, 2 * H], f16)    # Q   [l_chunk][l_in, h]
        kw16 = persist.tile([P, 2 * H], f16)
        vd16 = persist.tile([P, 2 * H], f16)    # V_down [l_chunk][l_in, h]
        vu16 = persist.tile([P, L], f16)        # V_up   [h, l]
        vu_bf = persist.tile([P, L], bf16)      # V_up as bf16 (out matmul)
        xt16 = [
            [
                persist.tile(
                    [P, 1024], f16, tag=f"xt{c}_{s}", name=f"xt16_{c}_{s}"
                )
                for s in range(4)
            ]
            for c in range(2)
        ]
        qT16 = persist.tile([P, N], f16)        # q.T       [h, n]
        kT16 = persist.tile([P, N], f16)        # k.T       [h, m]
        w_sb = persist.tile([P, MT * H], bf16)  # x@V_down  [m_tile][m_in, h]

        # ---------------- phase A: direct fp16 loads ----------------
        # the first x chunk (s=0) is the critical path for the first QK
        # tiles - issue it before anything else on the serial issue path
        def dma_xt(s):
            for c in range(2):
                for h2 in range(2):
                    nc.sync.dma_start(
                        xt16[c][s][:, h2 * 512:(h2 + 1) * 512],
                        xT_ext[
                            c * P:(c + 1) * P,
                            s * 1024 + h2 * 512: s * 1024 + (h2 + 1) * 512,
                        ],
                    )
        dma_xt(0)
        for c in range(2):
            nc.sync.dma_start(qw16[:, c * H:(c + 1) * H], wq_ext[c * P:(c + 1) * P, :])
            nc.sync.dma_start(kw16[:, c * H:(c + 1) * H], wk_ext[c * P:(c + 1) * P, :])
            nc.sync.dma_start(vd16[:, c * H:(c + 1) * H], vd_ext[c * P:(c + 1) * P, :])
        nc.sync.dma_start(vu16[:], vu_ext[:, :])
        nc.vector.tensor_copy(vu_bf[:], vu16[:])
        for s in range(1, 4):
            dma_xt(s)

        # ------------- phases B+C: projections fused with attention -------
        with (
            tc.tile_pool(name="est", bufs=36) as est_pool,
            tc.tile_pool(name="sb_small", bufs=4) as sb_small,
            tc.tile_pool(name="outfin", bufs=4) as outfin_pool,
            tc.tile_pool(name="stp", bufs=3, space="PSUM") as stp,
            tc.tile_pool(name="mtp", bufs=1, space="PSUM") as mtp,
        ):
            est = {}      # (k, mt) -> bf16 [128, NB] exp score tiles
            mtiles = {}   # k -> psum numerator mid^T [h, n] tile
            mscs = {}     # k -> normalized mid (f16, SBUF)
            bc = {}       # k -> [128, NB] f32 broadcast 1/rowsum
            tree = {}     # (k, level, i) -> partial rowsum tiles

            def proj_qkT(w16, dst, f):
                ps = stp.tile([P, NB], f32, tag="stp", name=f"pjp_{f}")
                for c in range(2):
                    nc.tensor.matmul(
                        ps[:, :NBH],
                        w16[:, c * H:(c + 1) * H],
                        xt16[c][f // 2][:, (f % 2) * NBH:(f % 2 + 1) * NBH],
                        start=(c == 0), stop=(c == 1),
                    )
                nc.vector.tensor_copy(dst[:, f * NBH:(f + 1) * NBH], ps[:, :NBH])

            def proj_w(mt):
                ps = stp.tile([P, NB], f32, tag="stp", name=f"pjw_{mt}")
                for c in range(2):
                    nc.tensor.matmul(
                        ps[:, :H],
                        xt16[c][mt // 8][:, (mt % 8) * P:(mt % 8 + 1) * P],
                        vd16[:, c * H:(c + 1) * H],
                        start=(c == 0), stop=(c == 1),
                    )
                nc.vector.tensor_copy(w_sb[:, mt * H:(mt + 1) * H], ps[:, :H])

            def qk_exp(k, mt):
                ps = stp.tile([P, NB], f32, tag="stp", name=f"qk_{k}_{mt}")
                for h in range(2):
                    nc.tensor.matmul(
                        ps[:, h * NBH:(h + 1) * NBH],
                        kT16[:, mt * P:(mt + 1) * P],
                        qT16[:, k * NB + h * NBH: k * NB + (h + 1) * NBH],
                        start=True, stop=True,
                    )
                e = est_pool.tile([P, NB], bf16, tag="est", name=f"est_{k}_{mt}")
                est[(k, mt)] = e
                nc.scalar.activation(e[:], ps[:], AF.Exp, scale=1.0 / SCALER)

            def tree_adds(k, mt):
                # lazily build the 5-level pairwise rowsum tree on DVE;
                # levels 4/5 accumulate in fp32
                if mt % 2 == 1:
                    t = sb_small.tile([P, NB], bf16, tag="t1", bufs=3,
                                      name=f"t1_{k}_{mt}")
                    nc.vector.tensor_add(t[:], est[(k, mt - 1)][:], est[(k, mt)][:])
                    tree[(k, 1, mt // 2)] = t
                if mt % 4 == 3:
                    t = sb_small.tile([P, NB], bf16, tag="t2", bufs=3,
                                      name=f"t2_{k}_{mt}")
                    nc.vector.tensor_add(
                        t[:], tree[(k, 1, mt // 2 - 1)][:], tree[(k, 1, mt // 2)][:]
                    )
                    tree[(k, 2, mt // 4)] = t
                if mt % 8 == 7:
                    t = sb_small.tile([P, NB], bf16, tag="t3", bufs=3,
                                      name=f"t3_{k}_{mt}")
                    nc.vector.tensor_add(
                        t[:], tree[(k, 2, mt // 4 - 1)][:], tree[(k, 2, mt // 4)][:]
                    )
                    tree[(k, 3, mt // 8)] = t
                if mt % 16 == 15:
                    t = sb_small.tile([P, NB], f32, tag="t4", bufs=2,
                                      name=f"t4_{k}_{mt}")
                    nc.vector.tensor_add(
                        t[:], tree[(k, 3, mt // 8 - 1)][:], tree[(k, 3, mt // 8)][:]
                    )
                    tree[(k, 4, mt // 16)] = t
                if mt == 31:
                    t = sb_small.tile([P, NB], f32, tag="t5", bufs=2,
                                      name=f"t5_{k}")
                    nc.vector.tensor_add(
                        t[:], tree[(k, 4, 0)][:], tree[(k, 4, 1)][:]
                    )
                    tree[(k, 5, 0)] = t

            def rowsum_finish(k):
                # all-reduce over partitions on GpSimd (systolic daisy chain,
                # broadcast result), then fast reciprocal on DVE
                rsb = sb_small.tile([P, NB], f32, tag="rsb", bufs=2,
                                    name=f"rsb_{k}")
                nc.gpsimd.partition_all_reduce(
                    rsb[:], tree[(k, 5, 0)][:], channels=P,
                    reduce_op=bass_isa.ReduceOp.add,
                )
                bc[k] = rsb

            def recip_bc(k):
                # deferred so the Vector FIFO never head-of-line blocks on
                # the GpSimd PartitionAllReduce (finished a while ago)
                bck = sb_small.tile([P, NB], f32, tag="bc", bufs=2,
                                    name=f"bc_{k}")
                nc.vector.reciprocal_approx_fast(bck[:], bc[k][:])
                bc[k] = bck

            def norm_mid(k):
                # plain copy on the Scalar engine: it rides the exp stream,
                # so the mid PSUM tile frees right on schedule and the next
                # block's first PV matmul never stalls. The 1/rowsum scale
                # moves to the fin stage (linear, commutes with V_up).
                msc = sb_small.tile([P, NB], bf16, tag="msc", bufs=2,
                                    name=f"msc_{k}")
                nc.scalar.activation(msc[:], mtiles[k][:], AF.Copy)
                mscs[k] = msc

            def drain_out(k):
                # apply V_up, normalize by 1/rowsum, store transposed (f16)
                for lt in range(2):
                    op = stp.tile([P, NB], f32, tag="stp", name=f"op_{k}_{lt}")
                    for h in range(2):
                        nc.tensor.matmul(
                            op[:, h * NBH:(h + 1) * NBH],
                            vu_bf[:, lt * P:(lt + 1) * P],
                            mscs[k][:, h * NBH:(h + 1) * NBH],
                            start=True, stop=True,
                        )
                    fin = outfin_pool.tile([P, NB], f16, tag="fin")
                    nc.vector.tensor_mul(fin[:], op[:], bc[k][:])
                    nc.gpsimd.dma_start(
                        out_ext[lt * P:(lt + 1) * P, k * NB:(k + 1) * NB],
                        fin[:],
                    )

            def pv2(kk, j, mid):
                for h in range(2):
                    nc.tensor.matmul(
                        mid[:, h * NBH:(h + 1) * NBH],
                        w_sb[:, j * H:(j + 1) * H],
                        est[(kk, j)][:, h * NBH:(h + 1) * NBH],
                        start=(j == 0), stop=(j == MT - 1),
                    )

            # PE warm-up: junk matmuls while the x DMA is in flight, so the
            # HAM clock gate is already at 2.4 GHz when real work starts
            wrm = persist.tile([P, NBH], bf16, name="wrm")
            nc.gpsimd.memset(wrm[:], 0.0)
            for i in range(46):
                ps = stp.tile([P, NB], f32, tag="stp", name=f"warm_{i}")
                nc.tensor.matmul(
                    ps[:, :NBH], wrm[:, :P], wrm[:], start=True, stop=True
                )

            # Uniform half-block-lagged schedule: during block k, PE runs
            # QK(k) plus the oldest pending attention@w work (last half of
            # block k-1, then first half of block k), so per-mt PE load is a
            # constant 4 matmuls and the Scalar engine's exp stream paces
            # everything. Block 0 uses the projection matmuls as its filler.
            # head: the first QK tiles need qT/kT half-blocks 0,1 (chunk s0)
            proj_qkT(qw16, qT16, 0)
            proj_qkT(qw16, qT16, 1)
            proj_qkT(kw16, kT16, 0)
            proj_qkT(kw16, kT16, 1)

            for k in range(NT):
                for mt in range(MT):
                    qk_exp(k, mt)
                    if k == 0:
                        # one w-projection per mt keeps w 16 tiles ahead of
                        # its consumer PV(0, mt-16)
                        proj_w(mt)
                        if mt % 4 == 0 and mt < 24:
                            proj_qkT(kw16, kT16, mt // 4 + 2)
                        if mt in (8, 10):
                            proj_qkT(qw16, qT16, (mt - 8) // 2 + 2)
                    if k == 1 and mt in (1, 3, 5, 7):
                        proj_qkT(qw16, qT16, (mt - 1) // 2 + 4)
                    if k >= 1 and mt <= 15:
                        pv2(k - 1, 16 + mt, mtiles[k - 1])
                    if mt == 16:
                        mid = mtp.tile([P, NB], f32, tag="mtp", name=f"mid_{k}")
                        mtiles[k] = mid
                    if mt >= 16:
                        pv2(k, mt - 16, mtiles[k])
                    if k >= 1 and mt == 10:
                        recip_bc(k - 1)
                    if k >= 1 and mt == 15:
                        norm_mid(k - 1)
                    tree_adds(k, mt)
                    if k >= 1 and mt == 18:
                        drain_out(k - 1)
                rowsum_finish(k)

            # epilogue: finish block 3's product and drain it
            recip_bc(NT - 1)
            k3 = NT - 1
            for h in range(2):
                for j in range(16, MT):
                    nc.tensor.matmul(
                        mtiles[k3][:, h * NBH:(h + 1) * NBH],
                        w_sb[:, j * H:(j + 1) * H],
                        est[(k3, j)][:, h * NBH:(h + 1) * NBH],
                        start=(j == 0), stop=(j == MT - 1),
                    )
                # drain this half as soon as its chain closes
                msc_h = sb_small.tile([P, NBH], bf16, tag="msch", bufs=2,
                                      name=f"msch_{h}")
                nc.scalar.activation(
                    msc_h[:], mtiles[k3][:, h * NBH:(h + 1) * NBH], AF.Copy
                )
                for lt in range(2):
                    op = stp.tile([P, NB], f32, tag="stp", name=f"ope_{h}_{lt}")
                    nc.tensor.matmul(
                        op[:, :NBH], vu_bf[:, lt * P:(lt + 1) * P], msc_h[:],
                        start=True, stop=True,
                    )
                    fin = outfin_pool.tile([P, NBH], f16, tag="fine", bufs=4)
                    nc.vector.tensor_mul(
                        fin[:], op[:, :NBH],
                        bc[k3][:, h * NBH:(h + 1) * NBH],
                    )
                    nc.gpsimd.dma_start(
                        out_ext[
                            lt * P:(lt + 1) * P,
                            k3 * NB + h * NBH: k3 * NB + (h + 1) * NBH,
                        ],
                        fin[:],
                    )

    if not nc.is_finalized():
        nc.finalize()
    return nc


_GRAPH_CACHE = {}


def _get_graph():
    if "nc" not in _GRAPH_CACHE:
        _GRAPH_CACHE["nc"] = _build()
    return _GRAPH_CACHE["nc"]


def run(inputs: dict, trace: bool = False):
    """Run the SPMD kernel on 8 cores. Returns (output, BassKernelResults)."""
    from concourse.bass_utils import run_bass_kernel_spmd

    x = np.asarray(inputs["x"], dtype=np.float32)
    Q = np.asarray(inputs["Q"], dtype=np.float32)[0]
    K = np.asarray(inputs["K"], dtype=np.float32)[0]
    Vd = np.asarray(inputs["V_down"], dtype=np.float32)[0]
    Vu = np.asarray(inputs["V_up"], dtype=np.float32)[0]

    wq = np.ascontiguousarray(Q).astype(np.float16)
    wk = np.ascontiguousarray(K).astype(np.float16)
    vd = np.ascontiguousarray(Vd).astype(np.float16)
    vu = np.ascontiguousarray(Vu).astype(np.float16)

    in_maps = []
    for b in range(B):
        in_maps.append({
            "xT": np.ascontiguousarray(x[b].T).astype(np.float16),
            "Wq": wq,
            "Wk": wk,
            "Vd": vd,
            "Vu": vu,
        })

    nc = _get_graph()
    res = run_bass_kernel_spmd(nc, in_maps, core_ids=list(range(B)), trace=trace)
    # device output is [L, N] per core; un-transpose during the gather
    out = np.stack([np.asarray(res.results[i]["out"]).astype(np.float32).T for i in range(B)])
    return np.ascontiguousarray(out, dtype=np.float32), res


def kernel(**inputs) -> np.ndarray:
    out, _ = run(inputs, trace=False)
    return out


# revision 39
# speedup vs baseline: 1.0039x; 1.0039x over previous
"""Trainium2 Bass kernel for nn_Attention_27358941675773.

Reference computation (per batch b):
    q = x @ Q              [N, H]
    k = x @ K              [N, H]
    V = V_down @ V_up      [L, L]
    v = x @ V              [N, L]
    S = q @ k.T / 256      [N, N]
    out = softmax(S) @ v   [N, L]

Sharding: pure data-parallel over batch B=8 across the 8 NeuronCores
(one batch element per core); small params replicated. No collectives.

Per-core kernel strategy (N=4096, L=256, H=128):
  - Inputs are shipped as fp16 (x transposed to [L, N]); all matmuls run
    at full PE rate (1 cyc/row). qT [H,N] and kT [H,N] are computed
    directly in transposed layout so scores can be built as S_T[m, n]
    (keys on partitions) without any transposes.
  - The value path is factored through the rank-H bottleneck:
        out = softmax(S) @ x @ V_down @ V_up
    so the O(N^2) matmul contracts into H=128 columns (w = x @ V_down),
    and V_up is applied after the softmax-normalization - halving the
    PE work of the attention*value product.
  - exp(S_T/256) runs on the Scalar engine straight out of PSUM, written
    as bf16 (scores reach ~±70; exp stays in fp32/bf16 range, so no
    max-subtraction pass is needed).
  - softmax denominator rowsum[n] = sum_m exp(S_T[m,n]): two levels of
    pairwise adds on the Vector engine, then an 8-chunk ones-vector
    matmul accumulated in PSUM (partition-axis reduction).
  - normalization: 1/rowsum is partition-broadcast on GpSimd and applied
    to mid^T = w^T-weighted numerator with one Vector multiply; the
    output is stored TRANSPOSED [L, N] in DRAM and un-transposed on the
    host during the gather.
  - The attention*w matmul of block k-1 is software-pipelined against
    the QK/exp of block k so the Scalar engine's exp stream stays hidden.
"""

import os
import sys

import numpy as np

for _p in ("/opt/trn_rl_repo",):
    if _p not in sys.path and os.path.isdir(_p):
        sys.path.insert(0, _p)

B, N, L, H = 8, 4096, 256, 128
SCALER = 256.0
NB = 1024           # query-block (free dim of score tiles)
NBH = 512           # half block (one PSUM bank of fp32)
NT = N // NB        # 4 query blocks
MT = N // 128       # 32 key tiles of 128
P = 128


def _build():
    import concourse.bass as bass
    import concourse.tile as tile
    from concourse import bacc, bass_isa, mybir
    from contextlib import ExitStack

    f32 = mybir.dt.float32
    f16 = mybir.dt.float16
    bf16 = mybir.dt.bfloat16
    AF = mybir.ActivationFunctionType

    nc = bacc.Bacc(
        "TRN2", target_bir_lowering=False, debug=False, num_devices=B
    )

    xT_ext = nc.declare_dram_parameter("xT", [L, N], f16, isOutput=False)
    wq_ext = nc.declare_dram_parameter("Wq", [L, H], f16, isOutput=False)
    wk_ext = nc.declare_dram_parameter("Wk", [L, H], f16, isOutput=False)
    vd_ext = nc.declare_dram_parameter("Vd", [L, H], f16, isOutput=False)
    vu_ext = nc.declare_dram_parameter("Vu", [H, L], f16, isOutput=False)
    # output stored transposed [L, N]; host un-transposes at gather
    out_ext = nc.declare_dram_parameter("out", [L, N], f16, isOutput=True)

    with tile.TileContext(nc) as tc, ExitStack() as ctx:
        persist = ctx.enter_context(tc.tile_pool(name="persist", bufs=1))

        ones_bf = persist.tile([P, 1], bf16)
        nc.gpsimd.memset(ones_bf[:], 1.0)
        # touch Exp right away so the ~2.7us ACT table load overlaps the
        # input DMAs instead of delaying the first real exp
        dum = persist.tile([1, 2], f32)
        nc.gpsimd.memset(dum[:], 0.0)
        nc.scalar.activation(dum[:, 1:2], dum[:, 0:1], AF.Exp)

        qw16 = persist.tile([P, 2 * H], f16)    # Q   [l_chunk][l_in, h]
        kw16 = persist.tile([P, 2 * H], f16)
        vd16 = persist.tile([P, 2 * H], f16)    # V_down [l_chunk][l_in, h]
        vu16 = persist.tile([P, L], f16)        # V_up   [h, l]
        vu_bf = persist.tile([P, L], bf16)      # V_up as bf16 (out matmul)
        xt16 = [
            [
                persist.tile(
                    [P, 1024], f16, tag=f"xt{c}_{s}", name=f"xt16_{c}_{s}"
                )
                for s in range(4)
            ]
            for c in range(2)
        ]
        qT16 = persist.tile([P, N], f16)        # q.T       [h, n]
        kT16 = persist.tile([P, N], f16)        # k.T       [h, m]
        w_sb = persist.tile([P, MT * H], bf16)  # x@V_down  [m_tile][m_in, h]

        # ---------------- phase A: direct fp16 loads ----------------
        # the first x chunk (s=0) is the critical path for the first QK
        # tiles - issue it before anything else on the serial issue path
        def dma_xt(s):
            for c in range(2):
                for h2 in range(2):
                    nc.sync.dma_start(
                        xt16[c][s][:, h2 * 512:(h2 + 1) * 512],
                        xT_ext[
                            c * P:(c + 1) * P,
                            s * 1024 + h2 * 512: s * 1024 + (h2 + 1) * 512,
                        ],
                    )
        dma_xt(0)
        for c in range(2):
            nc.sync.dma_start(qw16[:, c * H:(c + 1) * H], wq_ext[c * P:(c + 1) * P, :])
            nc.sync.dma_start(kw16[:, c * H:(c + 1) * H], wk_ext[c * P:(c + 1) * P, :])
            nc.sync.dma_start(vd16[:, c * H:(c + 1) * H], vd_ext[c * P:(c + 1) * P, :])
        nc.sync.dma_start(vu16[:], vu_ext[:, :])
        nc.vector.tensor_copy(vu_bf[:], vu16[:])
        for s in range(1, 4):
            dma_xt(s)

        # ------------- phases B+C: projections fused with attention -------
        with (
            tc.tile_pool(name="est", bufs=36) as est_pool,
            tc.tile_pool(name="sb_small", bufs=4) as sb_small,
            tc.tile_pool(name="outfin", bufs=4) as outfin_pool,
            tc.tile_pool(name="stp", bufs=3, space="PSUM") as stp,
            tc.tile_pool(name="mtp", bufs=1, space="PSUM") as mtp,
        ):
            est = {}      # (k, mt) -> bf16 [128, NB] exp score tiles
            mtiles = {}   # k -> psum numerator mid^T [h, n] tile
            mscs = {}     # k -> normalized mid (f16, SBUF)
            bc = {}       # k -> [128, NB] f32 broadcast 1/rowsum
            tree = {}     # (k, level, i) -> partial rowsum tiles

            def proj_qkT(w16, dst, f):
                ps = stp.tile([P, NB], f32, tag="stp", name=f"pjp_{f}")
                for c in range(2):
                    nc.tensor.matmul(
                        ps[:, :NBH],
                        w16[:, c * H:(c + 1) * H],
                        xt16[c][f // 2][:, (f % 2) * NBH:(f % 2 + 1) * NBH],
                        start=(c == 0), stop=(c == 1),
                    )
                nc.vector.tensor_copy(dst[:, f * NBH:(f + 1) * NBH], ps[:, :NBH])

            def proj_w(mt):
                ps = stp.tile([P, NB], f32, tag="stp", name=f"pjw_{mt}")
                for c in range(2):
                    nc.tensor.matmul(
                        ps[:, :H],
                        xt16[c][mt // 8][:, (mt % 8) * P:(mt % 8 + 1) * P],
                        vd16[:, c * H:(c + 1) * H],
                        start=(c == 0), stop=(c == 1),
                    )
                nc.vector.tensor_copy(w_sb[:, mt * H:(mt + 1) * H], ps[:, :H])

            def qk_exp(k, mt):
                ps = stp.tile([P, NB], f32, tag="stp", name=f"qk_{k}_{mt}")
                for h in range(2):
                    nc.tensor.matmul(
                        ps[:, h * NBH:(h + 1) * NBH],
                        kT16[:, mt * P:(mt + 1) * P],
                        qT16[:, k * NB + h * NBH: k * NB + (h + 1) * NBH],
                        start=True, stop=True,
                    )
                e = est_pool.tile([P, NB], bf16, tag="est", name=f"est_{k}_{mt}")
                est[(k, mt)] = e
                nc.scalar.activation(e[:], ps[:], AF.Exp, scale=1.0 / SCALER)

            def tree_adds(k, mt):
                # lazily build the 5-level pairwise rowsum tree on DVE;
                # levels 4/5 accumulate in fp32
                if mt % 2 == 1:
                    t = sb_small.tile([P, NB], bf16, tag="t1", bufs=3,
                                      name=f"t1_{k}_{mt}")
                    nc.vector.tensor_add(t[:], est[(k, mt - 1)][:], est[(k, mt)][:])
                    tree[(k, 1, mt // 2)] = t
                if mt % 4 == 3:
                    t = sb_small.tile([P, NB], bf16, tag="t2", bufs=3,
                                      name=f"t2_{k}_{mt}")
                    nc.vector.tensor_add(
                        t[:], tree[(k, 1, mt // 2 - 1)][:], tree[(k, 1, mt // 2)][:]
                    )
                    tree[(k, 2, mt // 4)] = t
                if mt % 8 == 7:
                    t = sb_small.tile([P, NB], bf16, tag="t3", bufs=3,
                                      name=f"t3_{k}_{mt}")
                    nc.vector.tensor_add(
                        t[:], tree[(k, 2, mt // 4 - 1)][:], tree[(k, 2, mt // 4)][:]
                    )
                    tree[(k, 3, mt // 8)] = t
                if mt % 16 == 15:
                    t = sb_small.tile([P, NB], f32, tag="t4", bufs=2,
                                      name=f"t4_{k}_{mt}")
                    nc.vector.tensor_add(
                        t[:], tree[(k, 3, mt // 8 - 1)][:], tree[(k, 3, mt // 8)][:]
                    )
                    tree[(k, 4, mt // 16)] = t
                if mt == 31:
                    t = sb_small.tile([P, NB], f32, tag="t5", bufs=2,
                                      name=f"t5_{k}")
                    nc.vector.tensor_add(
                        t[:], tree[(k, 4, 0)][:], tree[(k, 4, 1)][:]
                    )
                    tree[(k, 5, 0)] = t

            def rowsum_finish(k):
                # all-reduce over partitions on GpSimd (systolic daisy chain,
                # broadcast result), then fast reciprocal on DVE
                rsb = sb_small.tile([P, NB], f32, tag="rsb", bufs=2,
                                    name=f"rsb_{k}")
                nc.gpsimd.partition_all_reduce(
                    rsb[:], tree[(k, 5, 0)][:], channels=P,
                    reduce_op=bass_isa.ReduceOp.add,
                )
                bc[k] = rsb

            def recip_bc(k):
                # deferred so the Vector FIFO never head-of-line blocks on
                # the GpSimd PartitionAllReduce (finished a while ago)
                bck = sb_small.tile([P, NB], f32, tag="bc", bufs=2,
                                    name=f"bc_{k}")
                nc.vector.reciprocal_approx_fast(bck[:], bc[k][:])
                bc[k] = bck

            def norm_mid(k):
                # plain copy on the Scalar engine: it rides the exp stream,
                # so the mid PSUM tile frees right on schedule and the next
                # block's first PV matmul never stalls. The 1/rowsum scale
                # moves to the fin stage (linear, commutes with V_up).
                msc = sb_small.tile([P, NB], bf16, tag="msc", bufs=2,
                                    name=f"msc_{k}")
                nc.scalar.activation(msc[:], mtiles[k][:], AF.Copy)
                mscs[k] = msc

            def drain_out(k):
                # apply V_up, normalize by 1/rowsum, store transposed (f16)
                for lt in range(2):
                    op = stp.tile([P, NB], f32, tag="stp", name=f"op_{k}_{lt}")
                    for h in range(2):
                        nc.tensor.matmul(
                            op[:, h * NBH:(h + 1) * NBH],
                            vu_bf[:, lt * P:(lt + 1) * P],
                            mscs[k][:, h * NBH:(h + 1) * NBH],
                            start=True, stop=True,
                        )
                    fin = outfin_pool.tile([P, NB], f16, tag="fin")
                    nc.vector.tensor_mul(fin[:], op[:], bc[k][:])
                    nc.gpsimd.dma_start(
                        out_ext[lt * P:(lt + 1) * P, k * NB:(k + 1) * NB],
                        fin[:],
                    )

            def pv2(kk, j, mid):
                for h in range(2):
                    nc.tensor.matmul(
                        mid[:, h * NBH:(h + 1) * NBH],
                        w_sb[:, j * H:(j + 1) * H],
                        est[(kk, j)][:, h * NBH:(h + 1) * NBH],
                        start=(j == 0), stop=(j == MT - 1),
                    )

            # PE warm-up: junk matmuls while the x DMA is in flight, so the
            # HAM clock gate is already at 2.4 GHz when real work starts
            wrm = persist.tile([P, NBH], bf16, name="wrm")
            nc.vector.memset(wrm[:], 0.0)
            for i in range(52):
                ps = stp.tile([P, NB], f32, tag="stp", name=f"warm_{i}")
                nc.tensor.matmul(
                    ps[:, :NBH], wrm[:, :P], wrm[:], start=True, stop=True
                )

            # Uniform half-block-lagged schedule: during block k, PE runs
            # QK(k) plus the oldest pending attention@w work (last half of
            # block k-1, then first half of block k), so per-mt PE load is a
            # constant 4 matmuls and the Scalar engine's exp stream paces
            # everything. Block 0 uses the projection matmuls as its filler.
            # head: the first QK tiles need qT/kT half-blocks 0,1 (chunk s0)
            proj_qkT(qw16, qT16, 0)
            proj_qkT(qw16, qT16, 1)
            proj_qkT(kw16, kT16, 0)
            proj_qkT(kw16, kT16, 1)

            for k in range(NT):
                for mt in range(MT):
                    qk_exp(k, mt)
                    if k == 0:
                        # one w-projection per mt keeps w 16 tiles ahead of
                        # its consumer PV(0, mt-16)
                        proj_w(mt)
                        if mt % 4 == 0 and mt < 24:
                            proj_qkT(kw16, kT16, mt // 4 + 2)
                        if mt in (8, 10):
                            proj_qkT(qw16, qT16, (mt - 8) // 2 + 2)
                    if k == 1 and mt in (1, 3, 5, 7):
                        proj_qkT(qw16, qT16, (mt - 1) // 2 + 4)
                    if k >= 1 and mt <= 15:
                        pv2(k - 1, 16 + mt, mtiles[k - 1])
                    if mt == 16:
                        mid = mtp.tile([P, NB], f32, tag="mtp", name=f"mid_{k}")
                        mtiles[k] = mid
                    if mt >= 16:
                        pv2(k, mt - 16, mtiles[k])
                    if k >= 1 and mt == 10:
                        recip_bc(k - 1)
                    if k >= 1 and mt == 15:
                        norm_mid(k - 1)
                    tree_adds(k, mt)
                    if k >= 1 and mt == 18:
                        drain_out(k - 1)
                rowsum_finish(k)

            # epilogue: finish block 3's product and drain it
            recip_bc(NT - 1)
            k3 = NT - 1
            for h in range(2):
                for j in range(16, MT):
                    nc.tensor.matmul(
                        mtiles[k3][:, h * NBH:(h + 1) * NBH],
                        w_sb[:, j * H:(j + 1) * H],
                        est[(k3, j)][:, h * NBH:(h + 1) * NBH],
                        start=(j == 0), stop=(j == MT - 1),
                    )
                # drain this half as soon as its chain closes
                msc_h = sb_small.tile([P, NBH], bf16, tag="msch", bufs=2,
                                      name=f"msch_{h}")
                nc.scalar.activation(
                    msc_h[:], mtiles[k3][:, h * NBH:(h + 1) * NBH], AF.Copy
                )
                for lt in range(2):
                    op = stp.tile([P, NB], f32, tag="stp", name=f"ope_{h}_{lt}")
                    nc.tensor.matmul(
                        op[:, :NBH], vu_bf[:, lt * P:(lt + 1) * P], msc_h[:],
                        start=True, stop=True,
                    )
                    fin = outfin_pool.tile([P, NBH], f16, tag="fine", bufs=4)
                    nc.vector.tensor_mul(
                        fin[:], op[:, :NBH],
                        bc[k3][:, h * NBH:(h + 1) * NBH],
                    )
                    nc.gpsimd.dma_start(
                        out_ext[
                            lt * P:(lt + 1) * P,
                            k3 * NB + h * NBH: k3 * NB + (h + 1) * NBH,
                        ],
                        fin[:],
                    )

    if not nc.is_finalized():
        nc.finalize()
    return nc


_GRAPH_CACHE = {}


def _get_graph():
    if "nc" not in _GRAPH_CACHE:
        _GRAPH_CACHE["nc"] = _build()
    return _GRAPH_CACHE["nc"]


def run(inputs: dict, trace: bool = False):
    """Run the SPMD kernel on 8 cores. Returns (output, BassKernelResults)."""
    from concourse.bass_utils import run_bass_kernel_spmd

    x = np.asarray(inputs["x"], dtype=np.float32)
    Q = np.asarray(inputs["Q"], dtype=np.float32)[0]
    K = np.asarray(inputs["K"], dtype=np.float32)[0]
    Vd = np.asarray(inputs["V_down"], dtype=np.float32)[0]
    Vu = np.asarray(inputs["V_up"], dtype=np.float32)[0]

    wq = np.ascontiguousarray(Q).astype(np.float16)
    wk = np.ascontiguousarray(K).astype(np.float16)
    vd = np.ascontiguousarray(Vd).astype(np.float16)
    vu = np.ascontiguousarray(Vu).astype(np.float16)

    in_maps = []
    for b in range(B):
        in_maps.append({
            "xT": np.ascontiguousarray(x[b].T).astype(np.float16),
            "Wq": wq,
            "Wk": wk,
            "Vd": vd,
            "Vu": vu,
        })

    nc = _get_graph()
    res = run_bass_kernel_spmd(nc, in_maps, core_ids=list(range(B)), trace=trace)
    # device output is [L, N] per core; un-transpose during the gather
    out = np.stack([np.asarray(res.results[i]["out"]).astype(np.float32).T for i in range(B)])
    return np.ascontiguousarray(out, dtype=np.float32), res


def kernel(**inputs) -> np.ndarray:
    out, _ = run(inputs, trace=False)
    return out


# revision 40
# speedup vs baseline: 1.0053x; 1.0014x over previous
"""Trainium2 Bass kernel for nn_Attention_27358941675773.

Reference computation (per batch b):
    q = x @ Q              [N, H]
    k = x @ K              [N, H]
    V = V_down @ V_up      [L, L]
    v = x @ V              [N, L]
    S = q @ k.T / 256      [N, N]
    out = softmax(S) @ v   [N, L]

Sharding: pure data-parallel over batch B=8 across the 8 NeuronCores
(one batch element per core); small params replicated. No collectives.

Per-core kernel strategy (N=4096, L=256, H=128):
  - Inputs are shipped as fp16 (x transposed to [L, N]); all matmuls run
    at full PE rate (1 cyc/row). qT [H,N] and kT [H,N] are computed
    directly in transposed layout so scores can be built as S_T[m, n]
    (keys on partitions) without any transposes.
  - The value path is factored through the rank-H bottleneck:
        out = softmax(S) @ x @ V_down @ V_up
    so the O(N^2) matmul contracts into H=128 columns (w = x @ V_down),
    and V_up is applied after the softmax-normalization - halving the
    PE work of the attention*value product.
  - exp(S_T/256) runs on the Scalar engine straight out of PSUM, written
    as bf16 (scores reach ~±70; exp stays in fp32/bf16 range, so no
    max-subtraction pass is needed).
  - softmax denominator rowsum[n] = sum_m exp(S_T[m,n]): two levels of
    pairwise adds on the Vector engine, then an 8-chunk ones-vector
    matmul accumulated in PSUM (partition-axis reduction).
  - normalization: 1/rowsum is partition-broadcast on GpSimd and applied
    to mid^T = w^T-weighted numerator with one Vector multiply; the
    output is stored TRANSPOSED [L, N] in DRAM and un-transposed on the
    host during the gather.
  - The attention*w matmul of block k-1 is software-pipelined against
    the QK/exp of block k so the Scalar engine's exp stream stays hidden.
"""

import os
import sys

import numpy as np

for _p in ("/opt/trn_rl_repo",):
    if _p not in sys.path and os.path.isdir(_p):
        sys.path.insert(0, _p)

B, N, L, H = 8, 4096, 256, 128
SCALER = 256.0
NB = 1024           # query-block (free dim of score tiles)
NBH = 512           # half block (one PSUM bank of fp32)
NT = N // NB        # 4 query blocks
MT = N // 128       # 32 key tiles of 128
P = 128


def _build():
    import concourse.bass as bass
    import concourse.tile as tile
    from concourse import bacc, bass_isa, mybir
    from contextlib import ExitStack

    f32 = mybir.dt.float32
    f16 = mybir.dt.float16
    bf16 = mybir.dt.bfloat16
    AF = mybir.ActivationFunctionType

    nc = bacc.Bacc(
        "TRN2", target_bir_lowering=False, debug=False, num_devices=B
    )

    xT_ext = nc.declare_dram_parameter("xT", [L, N], f16, isOutput=False)
    wq_ext = nc.declare_dram_parameter("Wq", [L, H], f16, isOutput=False)
    wk_ext = nc.declare_dram_parameter("Wk", [L, H], f16, isOutput=False)
    vd_ext = nc.declare_dram_parameter("Vd", [L, H], f16, isOutput=False)
    vu_ext = nc.declare_dram_parameter("Vu", [H, L], f16, isOutput=False)
    # output stored transposed [L, N]; host un-transposes at gather
    out_ext = nc.declare_dram_parameter("out", [L, N], f16, isOutput=True)

    with tile.TileContext(nc) as tc, ExitStack() as ctx:
        persist = ctx.enter_context(tc.tile_pool(name="persist", bufs=1))

        ones_bf = persist.tile([P, 1], bf16)
        nc.gpsimd.memset(ones_bf[:], 1.0)
        # touch Exp right away so the ~2.7us ACT table load overlaps the
        # input DMAs instead of delaying the first real exp
        dum = persist.tile([1, 2], f32)
        nc.gpsimd.memset(dum[:], 0.0)
        nc.scalar.activation(dum[:, 1:2], dum[:, 0:1], AF.Exp)
        wrm = persist.tile([P, NBH], bf16, name="wrm")
        nc.vector.memset(wrm[:], 0.0)

        qw16 = persist.tile([P, 2 * H], f16)    # Q   [l_chunk][l_in, h]
        kw16 = persist.tile([P, 2 * H], f16)
        vd16 = persist.tile([P, 2 * H], f16)    # V_down [l_chunk][l_in, h]
        vu16 = persist.tile([P, L], f16)        # V_up   [h, l]
        vu_bf = persist.tile([P, L], bf16)      # V_up as bf16 (out matmul)
        xt16 = [
            [
                persist.tile(
                    [P, 1024], f16, tag=f"xt{c}_{s}", name=f"xt16_{c}_{s}"
                )
                for s in range(4)
            ]
            for c in range(2)
        ]
        qT16 = persist.tile([P, N], f16)        # q.T       [h, n]
        kT16 = persist.tile([P, N], f16)        # k.T       [h, m]
        w_sb = persist.tile([P, MT * H], bf16)  # x@V_down  [m_tile][m_in, h]

        # ---------------- phase A: direct fp16 loads ----------------
        # the first x chunk (s=0) is the critical path for the first QK
        # tiles - issue it before anything else on the serial issue path
        def dma_xt(s):
            for c in range(2):
                for h2 in range(2):
                    nc.sync.dma_start(
                        xt16[c][s][:, h2 * 512:(h2 + 1) * 512],
                        xT_ext[
                            c * P:(c + 1) * P,
                            s * 1024 + h2 * 512: s * 1024 + (h2 + 1) * 512,
                        ],
                    )
        dma_xt(0)
        for c in range(2):
            nc.sync.dma_start(qw16[:, c * H:(c + 1) * H], wq_ext[c * P:(c + 1) * P, :])
            nc.sync.dma_start(kw16[:, c * H:(c + 1) * H], wk_ext[c * P:(c + 1) * P, :])
            nc.sync.dma_start(vd16[:, c * H:(c + 1) * H], vd_ext[c * P:(c + 1) * P, :])
        nc.sync.dma_start(vu16[:], vu_ext[:, :])
        nc.vector.tensor_copy(vu_bf[:], vu16[:])
        for s in range(1, 4):
            dma_xt(s)

        # ------------- phases B+C: projections fused with attention -------
        with (
            tc.tile_pool(name="est", bufs=36) as est_pool,
            tc.tile_pool(name="sb_small", bufs=4) as sb_small,
            tc.tile_pool(name="outfin", bufs=4) as outfin_pool,
            tc.tile_pool(name="stp", bufs=3, space="PSUM") as stp,
            tc.tile_pool(name="mtp", bufs=1, space="PSUM") as mtp,
        ):
            est = {}      # (k, mt) -> bf16 [128, NB] exp score tiles
            mtiles = {}   # k -> psum numerator mid^T [h, n] tile
            mscs = {}     # k -> normalized mid (f16, SBUF)
            bc = {}       # k -> [128, NB] f32 broadcast 1/rowsum
            tree = {}     # (k, level, i) -> partial rowsum tiles

            def proj_qkT(w16, dst, f):
                ps = stp.tile([P, NB], f32, tag="stp", name=f"pjp_{f}")
                for c in range(2):
                    nc.tensor.matmul(
                        ps[:, :NBH],
                        w16[:, c * H:(c + 1) * H],
                        xt16[c][f // 2][:, (f % 2) * NBH:(f % 2 + 1) * NBH],
                        start=(c == 0), stop=(c == 1),
                    )
                nc.vector.tensor_copy(dst[:, f * NBH:(f + 1) * NBH], ps[:, :NBH])

            def proj_w(mt):
                ps = stp.tile([P, NB], f32, tag="stp", name=f"pjw_{mt}")
                for c in range(2):
                    nc.tensor.matmul(
                        ps[:, :H],
                        xt16[c][mt // 8][:, (mt % 8) * P:(mt % 8 + 1) * P],
                        vd16[:, c * H:(c + 1) * H],
                        start=(c == 0), stop=(c == 1),
                    )
                nc.vector.tensor_copy(w_sb[:, mt * H:(mt + 1) * H], ps[:, :H])

            def qk_exp(k, mt):
                ps = stp.tile([P, NB], f32, tag="stp", name=f"qk_{k}_{mt}")
                for h in range(2):
                    nc.tensor.matmul(
                        ps[:, h * NBH:(h + 1) * NBH],
                        kT16[:, mt * P:(mt + 1) * P],
                        qT16[:, k * NB + h * NBH: k * NB + (h + 1) * NBH],
                        start=True, stop=True,
                    )
                e = est_pool.tile([P, NB], bf16, tag="est", name=f"est_{k}_{mt}")
                est[(k, mt)] = e
                nc.scalar.activation(e[:], ps[:], AF.Exp, scale=1.0 / SCALER)

            def tree_adds(k, mt):
                # lazily build the 5-level pairwise rowsum tree on DVE;
                # levels 4/5 accumulate in fp32
                if mt % 2 == 1:
                    t = sb_small.tile([P, NB], bf16, tag="t1", bufs=3,
                                      name=f"t1_{k}_{mt}")
                    nc.vector.tensor_add(t[:], est[(k, mt - 1)][:], est[(k, mt)][:])
                    tree[(k, 1, mt // 2)] = t
                if mt % 4 == 3:
                    t = sb_small.tile([P, NB], bf16, tag="t2", bufs=3,
                                      name=f"t2_{k}_{mt}")
                    nc.vector.tensor_add(
                        t[:], tree[(k, 1, mt // 2 - 1)][:], tree[(k, 1, mt // 2)][:]
                    )
                    tree[(k, 2, mt // 4)] = t
                if mt % 8 == 7:
                    t = sb_small.tile([P, NB], bf16, tag="t3", bufs=3,
                                      name=f"t3_{k}_{mt}")
                    nc.vector.tensor_add(
                        t[:], tree[(k, 2, mt // 4 - 1)][:], tree[(k, 2, mt // 4)][:]
                    )
                    tree[(k, 3, mt // 8)] = t
                if mt % 16 == 15:
                    t = sb_small.tile([P, NB], f32, tag="t4", bufs=2,
                                      name=f"t4_{k}_{mt}")
                    nc.vector.tensor_add(
                        t[:], tree[(k, 3, mt // 8 - 1)][:], tree[(k, 3, mt // 8)][:]
                    )
                    tree[(k, 4, mt // 16)] = t
                if mt == 31:
                    t = sb_small.tile([P, NB], f32, tag="t5", bufs=2,
                                      name=f"t5_{k}")
                    nc.vector.tensor_add(
                        t[:], tree[(k, 4, 0)][:], tree[(k, 4, 1)][:]
                    )
                    tree[(k, 5, 0)] = t

            def rowsum_finish(k):
                # all-reduce over partitions on GpSimd (systolic daisy chain,
                # broadcast result), then fast reciprocal on DVE
                rsb = sb_small.tile([P, NB], f32, tag="rsb", bufs=2,
                                    name=f"rsb_{k}")
                nc.gpsimd.partition_all_reduce(
                    rsb[:], tree[(k, 5, 0)][:], channels=P,
                    reduce_op=bass_isa.ReduceOp.add,
                )
                bc[k] = rsb

            def recip_bc(k):
                # deferred so the Vector FIFO never head-of-line blocks on
                # the GpSimd PartitionAllReduce (finished a while ago)
                bck = sb_small.tile([P, NB], f32, tag="bc", bufs=2,
                                    name=f"bc_{k}")
                nc.vector.reciprocal_approx_fast(bck[:], bc[k][:])
                bc[k] = bck

            def norm_mid(k):
                # plain copy on the Scalar engine: it rides the exp stream,
                # so the mid PSUM tile frees right on schedule and the next
                # block's first PV matmul never stalls. The 1/rowsum scale
                # moves to the fin stage (linear, commutes with V_up).
                msc = sb_small.tile([P, NB], bf16, tag="msc", bufs=2,
                                    name=f"msc_{k}")
                nc.scalar.activation(msc[:], mtiles[k][:], AF.Copy)
                mscs[k] = msc

            def drain_out(k):
                # apply V_up, normalize by 1/rowsum, store transposed (f16)
                for lt in range(2):
                    op = stp.tile([P, NB], f32, tag="stp", name=f"op_{k}_{lt}")
                    for h in range(2):
                        nc.tensor.matmul(
                            op[:, h * NBH:(h + 1) * NBH],
                            vu_bf[:, lt * P:(lt + 1) * P],
                            mscs[k][:, h * NBH:(h + 1) * NBH],
                            start=True, stop=True,
                        )
                    fin = outfin_pool.tile([P, NB], f16, tag="fin")
                    nc.vector.tensor_mul(fin[:], op[:], bc[k][:])
                    nc.gpsimd.dma_start(
                        out_ext[lt * P:(lt + 1) * P, k * NB:(k + 1) * NB],
                        fin[:],
                    )

            def pv2(kk, j, mid):
                for h in range(2):
                    nc.tensor.matmul(
                        mid[:, h * NBH:(h + 1) * NBH],
                        w_sb[:, j * H:(j + 1) * H],
                        est[(kk, j)][:, h * NBH:(h + 1) * NBH],
                        start=(j == 0), stop=(j == MT - 1),
                    )

            # PE warm-up: junk matmuls while the x DMA is in flight, so the
            # HAM clock gate is already at 2.4 GHz when real work starts
            for i in range(45):
                ps = stp.tile([P, NB], f32, tag="stp", name=f"warm_{i}")
                nc.tensor.matmul(
                    ps[:, :NBH], wrm[:, :P], wrm[:], start=True, stop=True
                )

            # Uniform half-block-lagged schedule: during block k, PE runs
            # QK(k) plus the oldest pending attention@w work (last half of
            # block k-1, then first half of block k), so per-mt PE load is a
            # constant 4 matmuls and the Scalar engine's exp stream paces
            # everything. Block 0 uses the projection matmuls as its filler.
            # head: the first QK tiles need qT/kT half-blocks 0,1 (chunk s0)
            proj_qkT(qw16, qT16, 0)
            proj_qkT(qw16, qT16, 1)
            proj_qkT(kw16, kT16, 0)
            proj_qkT(kw16, kT16, 1)

            for k in range(NT):
                for mt in range(MT):
                    qk_exp(k, mt)
                    if k == 0:
                        # one w-projection per mt keeps w 16 tiles ahead of
                        # its consumer PV(0, mt-16)
                        proj_w(mt)
                        if mt % 4 == 0 and mt < 24:
                            proj_qkT(kw16, kT16, mt // 4 + 2)
                        if mt in (8, 10):
                            proj_qkT(qw16, qT16, (mt - 8) // 2 + 2)
                    if k == 1 and mt in (1, 3, 5, 7):
                        proj_qkT(qw16, qT16, (mt - 1) // 2 + 4)
                    if k >= 1 and mt <= 15:
                        pv2(k - 1, 16 + mt, mtiles[k - 1])
                    if mt == 16:
                        mid = mtp.tile([P, NB], f32, tag="mtp", name=f"mid_{k}")
                        mtiles[k] = mid
                    if mt >= 16:
                        pv2(k, mt - 16, mtiles[k])
                    if k >= 1 and mt == 10:
                        recip_bc(k - 1)
                    if k >= 1 and mt == 15:
                        norm_mid(k - 1)
                    tree_adds(k, mt)
                    if k >= 1 and mt == 18:
                        drain_out(k - 1)
                rowsum_finish(k)

            # epilogue: finish block 3's product and drain it
            recip_bc(NT - 1)
            k3 = NT - 1
            for h in range(2):
                for j in range(16, MT):
                    nc.tensor.matmul(
                        mtiles[k3][:, h * NBH:(h + 1) * NBH],
                        w_sb[:, j * H:(j + 1) * H],
                        est[(k3, j)][:, h * NBH:(h + 1) * NBH],
                        start=(j == 0), stop=(j == MT - 1),
                    )
                # drain this half as soon as its chain closes
                msc_h = sb_small.tile([P, NBH], bf16, tag="msch", bufs=2,
                                      name=f"msch_{h}")
                nc.scalar.activation(
                    msc_h[:], mtiles[k3][:, h * NBH:(h + 1) * NBH], AF.Copy
                )
                for lt in range(2):
                    op = stp.tile([P, NB], f32, tag="stp", name=f"ope_{h}_{lt}")
                    nc.tensor.matmul(
                        op[:, :NBH], vu_bf[:, lt * P:(lt + 1) * P], msc_h[:],
                        start=True, stop=True,
                    )
                    fin = outfin_pool.tile([P, NBH], f16, tag="fine", bufs=4)
                    nc.vector.tensor_mul(
                        fin[:], op[:, :NBH],
                        bc[k3][:, h * NBH:(h + 1) * NBH],
                    )
                    nc.gpsimd.dma_start(
                        out_ext[
                            lt * P:(lt + 1) * P,
                            k3 * NB + h * NBH: k3 * NB + (h + 1) * NBH,
                        ],
                        fin[:],
                    )

    if not nc.is_finalized():
        nc.finalize()
    return nc


_GRAPH_CACHE = {}


def _get_graph():
    if "nc" not in _GRAPH_CACHE:
        _GRAPH_CACHE["nc"] = _build()
    return _GRAPH_CACHE["nc"]


def run(inputs: dict, trace: bool = False):
    """Run the SPMD kernel on 8 cores. Returns (output, BassKernelResults)."""
    from concourse.bass_utils import run_bass_kernel_spmd

    x = np.asarray(inputs["x"], dtype=np.float32)
    Q = np.asarray(inputs["Q"], dtype=np.float32)[0]
    K = np.asarray(inputs["K"], dtype=np.float32)[0]
    Vd = np.asarray(inputs["V_down"], dtype=np.float32)[0]
    Vu = np.asarray(inputs["V_up"], dtype=np.float32)[0]

    wq = np.ascontiguousarray(Q).astype(np.float16)
    wk = np.ascontiguousarray(K).astype(np.float16)
    vd = np.ascontiguousarray(Vd).astype(np.float16)
    vu = np.ascontiguousarray(Vu).astype(np.float16)

    in_maps = []
    for b in range(B):
        in_maps.append({
            "xT": np.ascontiguousarray(x[b].T).astype(np.float16),
            "Wq": wq,
            "Wk": wk,
            "Vd": vd,
            "Vu": vu,
        })

    nc = _get_graph()
    res = run_bass_kernel_spmd(nc, in_maps, core_ids=list(range(B)), trace=trace)
    # device output is [L, N] per core; un-transpose during the gather
    out = np.stack([np.asarray(res.results[i]["out"]).astype(np.float32).T for i in range(B)])
    return np.ascontiguousarray(out, dtype=np.float32), res


def kernel(**inputs) -> np.ndarray:
    out, _ = run(inputs, trace=False)
    return out


# revision 41
# speedup vs baseline: 1.0306x; 1.0252x over previous
"""Trainium2 Bass kernel for nn_Attention_27358941675773.

Reference computation (per batch b):
    q = x @ Q              [N, H]
    k = x @ K              [N, H]
    V = V_down @ V_up      [L, L]
    v = x @ V              [N, L]
    S = q @ k.T / 256      [N, N]
    out = softmax(S) @ v   [N, L]

Sharding: pure data-parallel over batch B=8 across the 8 NeuronCores
(one batch element per core); small params replicated. No collectives.

Per-core kernel strategy (N=4096, L=256, H=128):
  - Inputs are shipped as fp16 (x transposed to [L, N]); all matmuls run
    at full PE rate (1 cyc/row). qT [H,N] and kT [H,N] are computed
    directly in transposed layout so scores can be built as S_T[m, n]
    (keys on partitions) without any transposes.
  - The value path is factored through the rank-H bottleneck:
        out = softmax(S) @ x @ V_down @ V_up
    so the O(N^2) matmul contracts into H=128 columns (w = x @ V_down),
    and V_up is applied after the softmax-normalization - halving the
    PE work of the attention*value product.
  - exp(S_T/256) runs on the Scalar engine straight out of PSUM, written
    as bf16 (scores reach ~±70; exp stays in fp32/bf16 range, so no
    max-subtraction pass is needed).
  - softmax denominator rowsum[n] = sum_m exp(S_T[m,n]): two levels of
    pairwise adds on the Vector engine, then an 8-chunk ones-vector
    matmul accumulated in PSUM (partition-axis reduction).
  - normalization: 1/rowsum is partition-broadcast on GpSimd and applied
    to mid^T = w^T-weighted numerator with one Vector multiply; the
    output is stored TRANSPOSED [L, N] in DRAM and un-transposed on the
    host during the gather.
  - The attention*w matmul of block k-1 is software-pipelined against
    the QK/exp of block k so the Scalar engine's exp stream stays hidden.
"""

import os
import sys

import numpy as np

for _p in ("/opt/trn_rl_repo",):
    if _p not in sys.path and os.path.isdir(_p):
        sys.path.insert(0, _p)

B, N, L, H = 8, 4096, 256, 128
SCALER = 256.0
NB = 1024           # query-block (free dim of score tiles)
NBH = 512           # half block (one PSUM bank of fp32)
NT = N // NB        # 4 query blocks
MT = N // 128       # 32 key tiles of 128
P = 128


def _build():
    import concourse.bass as bass
    import concourse.tile as tile
    from concourse import bacc, bass_isa, mybir
    from contextlib import ExitStack

    f32 = mybir.dt.float32
    f16 = mybir.dt.float16
    bf16 = mybir.dt.bfloat16
    AF = mybir.ActivationFunctionType

    nc = bacc.Bacc(
        "TRN2", target_bir_lowering=False, debug=False, num_devices=B
    )

    xT_ext = nc.declare_dram_parameter("xT", [L, N], f16, isOutput=False)
    wq_ext = nc.declare_dram_parameter("Wq", [L, H], f16, isOutput=False)
    wk_ext = nc.declare_dram_parameter("Wk", [L, H], f16, isOutput=False)
    vd_ext = nc.declare_dram_parameter("Vd", [L, H], f16, isOutput=False)
    vu_ext = nc.declare_dram_parameter("Vu", [H, L], f16, isOutput=False)
    # output stored transposed [L, N]; host un-transposes at gather
    out_ext = nc.declare_dram_parameter("out", [L, N], f16, isOutput=True)

    with tile.TileContext(nc) as tc, ExitStack() as ctx:
        persist = ctx.enter_context(tc.tile_pool(name="persist", bufs=1))

        ones_bf = persist.tile([P, 1], bf16)
        nc.gpsimd.memset(ones_bf[:], 1.0)
        # touch Exp right away so the ~2.7us ACT table load overlaps the
        # input DMAs instead of delaying the first real exp
        dum = persist.tile([1, 2], f32)
        nc.gpsimd.memset(dum[:], 0.0)
        nc.scalar.activation(dum[:, 1:2], dum[:, 0:1], AF.Exp)
        wrm = persist.tile([P, NBH], bf16, name="wrm")
        nc.vector.memset(wrm[:], 0.0)

        qw16 = persist.tile([P, 2 * H], f16)    # Q   [l_chunk][l_in, h]
        kw16 = persist.tile([P, 2 * H], f16)
        vd16 = persist.tile([P, 2 * H], f16)    # V_down [l_chunk][l_in, h]
        vu16 = persist.tile([P, L], f16)        # V_up   [h, l]
        vu_bf = persist.tile([P, L], bf16)      # V_up as bf16 (out matmul)
        xt16 = [
            [
                persist.tile(
                    [P, 1024], f16, tag=f"xt{c}_{s}", name=f"xt16_{c}_{s}"
                )
                for s in range(4)
            ]
            for c in range(2)
        ]
        qT16 = persist.tile([P, N], f16)        # q.T       [h, n]
        kT16 = persist.tile([P, N], f16)        # k.T       [h, m]
        w_sb = persist.tile([P, MT * H], bf16)  # x@V_down  [m_tile][m_in, h]

        # ---------------- phase A: direct fp16 loads ----------------
        # the first x chunk (s=0) is the critical path for the first QK
        # tiles - issue it before anything else on the serial issue path
        def dma_xt(s):
            for c in range(2):
                for h2 in range(2):
                    nc.sync.dma_start(
                        xt16[c][s][:, h2 * 512:(h2 + 1) * 512],
                        xT_ext[
                            c * P:(c + 1) * P,
                            s * 1024 + h2 * 512: s * 1024 + (h2 + 1) * 512,
                        ],
                    )
        dma_xt(0)
        for c in range(2):
            nc.sync.dma_start(qw16[:, c * H:(c + 1) * H], wq_ext[c * P:(c + 1) * P, :])
            nc.sync.dma_start(kw16[:, c * H:(c + 1) * H], wk_ext[c * P:(c + 1) * P, :])
            nc.sync.dma_start(vd16[:, c * H:(c + 1) * H], vd_ext[c * P:(c + 1) * P, :])
        nc.sync.dma_start(vu16[:], vu_ext[:, :])
        nc.vector.tensor_copy(vu_bf[:], vu16[:])
        for s in range(1, 4):
            dma_xt(s)

        # ------------- phases B+C: projections fused with attention -------
        with (
            tc.tile_pool(name="est", bufs=36) as est_pool,
            tc.tile_pool(name="sb_small", bufs=4) as sb_small,
            tc.tile_pool(name="outfin", bufs=4) as outfin_pool,
            tc.tile_pool(name="stp", bufs=3, space="PSUM") as stp,
            tc.tile_pool(name="mtp", bufs=1, space="PSUM") as mtp,
        ):
            est = {}      # (k, mt) -> bf16 [128, NB] exp score tiles
            mtiles = {}   # k -> psum numerator mid^T [h, n] tile
            mscs = {}     # k -> normalized mid (f16, SBUF)
            bc = {}       # k -> [128, NB] f32 broadcast 1/rowsum
            tree = {}     # (k, level, i) -> partial rowsum tiles

            def proj_qkT(w16, dst, f):
                ps = stp.tile([P, NB], f32, tag="stp", name=f"pjp_{f}")
                for c in range(2):
                    nc.tensor.matmul(
                        ps[:, :NBH],
                        w16[:, c * H:(c + 1) * H],
                        xt16[c][f // 2][:, (f % 2) * NBH:(f % 2 + 1) * NBH],
                        start=(c == 0), stop=(c == 1),
                    )
                nc.vector.tensor_copy(dst[:, f * NBH:(f + 1) * NBH], ps[:, :NBH])

            def proj_w(mt):
                ps = stp.tile([P, NB], f32, tag="stp", name=f"pjw_{mt}")
                for c in range(2):
                    nc.tensor.matmul(
                        ps[:, :H],
                        xt16[c][mt // 8][:, (mt % 8) * P:(mt % 8 + 1) * P],
                        vd16[:, c * H:(c + 1) * H],
                        start=(c == 0), stop=(c == 1),
                    )
                nc.vector.tensor_copy(w_sb[:, mt * H:(mt + 1) * H], ps[:, :H])

            def qk_exp(k, mt):
                ps = stp.tile([P, NB], f32, tag="stp", name=f"qk_{k}_{mt}")
                for h in range(2):
                    nc.tensor.matmul(
                        ps[:, h * NBH:(h + 1) * NBH],
                        kT16[:, mt * P:(mt + 1) * P],
                        qT16[:, k * NB + h * NBH: k * NB + (h + 1) * NBH],
                        start=True, stop=True,
                    )
                e = est_pool.tile([P, NB], bf16, tag="est", name=f"est_{k}_{mt}")
                est[(k, mt)] = e
                nc.scalar.activation(e[:], ps[:], AF.Exp, scale=1.0 / SCALER)

            def tree_adds(k, mt):
                # lazily build the 5-level pairwise rowsum tree on DVE;
                # levels 4/5 accumulate in fp32
                if mt % 2 == 1:
                    t = sb_small.tile([P, NB], bf16, tag="t1", bufs=3,
                                      name=f"t1_{k}_{mt}")
                    nc.vector.tensor_add(t[:], est[(k, mt - 1)][:], est[(k, mt)][:])
                    tree[(k, 1, mt // 2)] = t
                if mt % 4 == 3:
                    t = sb_small.tile([P, NB], bf16, tag="t2", bufs=3,
                                      name=f"t2_{k}_{mt}")
                    nc.vector.tensor_add(
                        t[:], tree[(k, 1, mt // 2 - 1)][:], tree[(k, 1, mt // 2)][:]
                    )
                    tree[(k, 2, mt // 4)] = t
                if mt % 8 == 7:
                    t = sb_small.tile([P, NB], bf16, tag="t3", bufs=3,
                                      name=f"t3_{k}_{mt}")
                    nc.vector.tensor_add(
                        t[:], tree[(k, 2, mt // 4 - 1)][:], tree[(k, 2, mt // 4)][:]
                    )
                    tree[(k, 3, mt // 8)] = t
                if mt % 16 == 15:
                    t = sb_small.tile([P, NB], f32, tag="t4", bufs=2,
                                      name=f"t4_{k}_{mt}")
                    nc.vector.tensor_add(
                        t[:], tree[(k, 3, mt // 8 - 1)][:], tree[(k, 3, mt // 8)][:]
                    )
                    tree[(k, 4, mt // 16)] = t
                if mt == 31:
                    t = sb_small.tile([P, NB], f32, tag="t5", bufs=2,
                                      name=f"t5_{k}")
                    nc.vector.tensor_add(
                        t[:], tree[(k, 4, 0)][:], tree[(k, 4, 1)][:]
                    )
                    tree[(k, 5, 0)] = t

            def rowsum_finish(k):
                # all-reduce over partitions on GpSimd (systolic daisy chain,
                # broadcast result), then fast reciprocal on DVE
                rsb = sb_small.tile([P, NB], f32, tag="rsb", bufs=2,
                                    name=f"rsb_{k}")
                nc.gpsimd.partition_all_reduce(
                    rsb[:], tree[(k, 5, 0)][:], channels=P,
                    reduce_op=bass_isa.ReduceOp.add,
                )
                bc[k] = rsb

            def recip_bc(k):
                # deferred so the Vector FIFO never head-of-line blocks on
                # the GpSimd PartitionAllReduce (finished a while ago)
                bck = sb_small.tile([P, NB], f32, tag="bc", bufs=2,
                                    name=f"bc_{k}")
                nc.vector.reciprocal_approx_fast(bck[:], bc[k][:])
                bc[k] = bck

            def norm_mid(k):
                # plain copy on the Scalar engine: it rides the exp stream,
                # so the mid PSUM tile frees right on schedule and the next
                # block's first PV matmul never stalls. The 1/rowsum scale
                # moves to the fin stage (linear, commutes with V_up).
                msc = sb_small.tile([P, NB], bf16, tag="msc", bufs=2,
                                    name=f"msc_{k}")
                nc.scalar.activation(msc[:], mtiles[k][:], AF.Copy)
                mscs[k] = msc

            def drain_out(k):
                # apply V_up, normalize by 1/rowsum, store transposed (f16)
                for lt in range(2):
                    op = stp.tile([P, NB], f32, tag="stp", name=f"op_{k}_{lt}")
                    for h in range(2):
                        nc.tensor.matmul(
                            op[:, h * NBH:(h + 1) * NBH],
                            vu_bf[:, lt * P:(lt + 1) * P],
                            mscs[k][:, h * NBH:(h + 1) * NBH],
                            start=True, stop=True,
                        )
                    fin = outfin_pool.tile([P, NB], f16, tag="fin")
                    nc.vector.tensor_mul(fin[:], op[:], bc[k][:])
                    nc.gpsimd.dma_start(
                        out_ext[lt * P:(lt + 1) * P, k * NB:(k + 1) * NB],
                        fin[:],
                    )

            def pv2(kk, j, mid):
                for h in range(2):
                    nc.tensor.matmul(
                        mid[:, h * NBH:(h + 1) * NBH],
                        w_sb[:, j * H:(j + 1) * H],
                        est[(kk, j)][:, h * NBH:(h + 1) * NBH],
                        start=(j == 0), stop=(j == MT - 1),
                    )

            # PE warm-up: junk matmuls while the x DMA is in flight, so the
            # HAM clock gate is already at 2.4 GHz when real work starts
            for i in range(16):
                ps = stp.tile([P, NB], f32, tag="stp", name=f"warm_{i}")
                nc.tensor.matmul(
                    ps[:, :NBH], wrm[:, :P], wrm[:], start=True, stop=True
                )

            # Uniform half-block-lagged schedule: during block k, PE runs
            # QK(k) plus the oldest pending attention@w work (last half of
            # block k-1, then first half of block k), so per-mt PE load is a
            # constant 4 matmuls and the Scalar engine's exp stream paces
            # everything. Block 0 uses the projection matmuls as its filler.
            # head: the first QK tiles need qT/kT half-blocks 0,1 (chunk s0)
            proj_qkT(qw16, qT16, 0)
            proj_qkT(qw16, qT16, 1)
            proj_qkT(kw16, kT16, 0)
            proj_qkT(kw16, kT16, 1)

            for k in range(NT):
                for mt in range(MT):
                    qk_exp(k, mt)
                    if k == 0:
                        # one w-projection per mt keeps w 16 tiles ahead of
                        # its consumer PV(0, mt-16)
                        proj_w(mt)
                        if mt % 4 == 0 and mt < 24:
                            proj_qkT(kw16, kT16, mt // 4 + 2)
                        if mt in (8, 10):
                            proj_qkT(qw16, qT16, (mt - 8) // 2 + 2)
                    if k == 1 and mt in (1, 3, 5, 7):
                        proj_qkT(qw16, qT16, (mt - 1) // 2 + 4)
                    if k >= 1 and mt <= 15:
                        pv2(k - 1, 16 + mt, mtiles[k - 1])
                    if mt == 16:
                        mid = mtp.tile([P, NB], f32, tag="mtp", name=f"mid_{k}")
                        mtiles[k] = mid
                    if mt >= 16:
                        pv2(k, mt - 16, mtiles[k])
                    if k >= 1 and mt == 10:
                        recip_bc(k - 1)
                    if k >= 1 and mt == 15:
                        norm_mid(k - 1)
                    tree_adds(k, mt)
                    if k >= 1 and mt == 18:
                        drain_out(k - 1)
                rowsum_finish(k)

            # epilogue: finish block 3's product and drain it
            recip_bc(NT - 1)
            k3 = NT - 1
            for h in range(2):
                for j in range(16, MT):
                    nc.tensor.matmul(
                        mtiles[k3][:, h * NBH:(h + 1) * NBH],
                        w_sb[:, j * H:(j + 1) * H],
                        est[(k3, j)][:, h * NBH:(h + 1) * NBH],
                        start=(j == 0), stop=(j == MT - 1),
                    )
                # drain this half as soon as its chain closes
                msc_h = sb_small.tile([P, NBH], bf16, tag="msch", bufs=2,
                                      name=f"msch_{h}")
                nc.scalar.activation(
                    msc_h[:], mtiles[k3][:, h * NBH:(h + 1) * NBH], AF.Copy
                )
                for lt in range(2):
                    op = stp.tile([P, NB], f32, tag="stp", name=f"ope_{h}_{lt}")
                    nc.tensor.matmul(
                        op[:, :NBH], vu_bf[:, lt * P:(lt + 1) * P], msc_h[:],
                        start=True, stop=True,
                    )
                    fin = outfin_pool.tile([P, NBH], f16, tag="fine", bufs=4)
                    nc.vector.tensor_mul(
                        fin[:], op[:, :NBH],
                        bc[k3][:, h * NBH:(h + 1) * NBH],
                    )
                    nc.gpsimd.dma_start(
                        out_ext[
                            lt * P:(lt + 1) * P,
                            k3 * NB + h * NBH: k3 * NB + (h + 1) * NBH,
                        ],
                        fin[:],
                    )

    if not nc.is_finalized():
        nc.finalize()
    return nc


_GRAPH_CACHE = {}


def _get_graph():
    if "nc" not in _GRAPH_CACHE:
        _GRAPH_CACHE["nc"] = _build()
    return _GRAPH_CACHE["nc"]


def run(inputs: dict, trace: bool = False):
    """Run the SPMD kernel on 8 cores. Returns (output, BassKernelResults)."""
    from concourse.bass_utils import run_bass_kernel_spmd

    x = np.asarray(inputs["x"], dtype=np.float32)
    Q = np.asarray(inputs["Q"], dtype=np.float32)[0]
    K = np.asarray(inputs["K"], dtype=np.float32)[0]
    Vd = np.asarray(inputs["V_down"], dtype=np.float32)[0]
    Vu = np.asarray(inputs["V_up"], dtype=np.float32)[0]

    wq = np.ascontiguousarray(Q).astype(np.float16)
    wk = np.ascontiguousarray(K).astype(np.float16)
    vd = np.ascontiguousarray(Vd).astype(np.float16)
    vu = np.ascontiguousarray(Vu).astype(np.float16)

    in_maps = []
    for b in range(B):
        in_maps.append({
            "xT": np.ascontiguousarray(x[b].T).astype(np.float16),
            "Wq": wq,
            "Wk": wk,
            "Vd": vd,
            "Vu": vu,
        })

    nc = _get_graph()
    res = run_bass_kernel_spmd(nc, in_maps, core_ids=list(range(B)), trace=trace)
    # device output is [L, N] per core; un-transpose during the gather
    out = np.stack([np.asarray(res.results[i]["out"]).astype(np.float32).T for i in range(B)])
    return np.ascontiguousarray(out, dtype=np.float32), res


def kernel(**inputs) -> np.ndarray:
    out, _ = run(inputs, trace=False)
    return out


# revision 42
# speedup vs baseline: 1.0334x; 1.0027x over previous
"""Trainium2 Bass kernel for nn_Attention_27358941675773.

Reference computation (per batch b):
    q = x @ Q              [N, H]
    k = x @ K              [N, H]
    V = V_down @ V_up      [L, L]
    v = x @ V              [N, L]
    S = q @ k.T / 256      [N, N]
    out = softmax(S) @ v   [N, L]

Sharding: pure data-parallel over batch B=8 across the 8 NeuronCores
(one batch element per core); small params replicated. No collectives.

Per-core kernel strategy (N=4096, L=256, H=128):
  - Inputs are shipped as fp16 (x transposed to [L, N]); all matmuls run
    at full PE rate (1 cyc/row). qT [H,N] and kT [H,N] are computed
    directly in transposed layout so scores can be built as S_T[m, n]
    (keys on partitions) without any transposes.
  - The value path is factored through the rank-H bottleneck:
        out = softmax(S) @ x @ V_down @ V_up
    so the O(N^2) matmul contracts into H=128 columns (w = x @ V_down),
    and V_up is applied after the softmax-normalization - halving the
    PE work of the attention*value product.
  - exp(S_T/256) runs on the Scalar engine straight out of PSUM, written
    as bf16 (scores reach ~±70; exp stays in fp32/bf16 range, so no
    max-subtraction pass is needed).
  - softmax denominator rowsum[n] = sum_m exp(S_T[m,n]): two levels of
    pairwise adds on the Vector engine, then an 8-chunk ones-vector
    matmul accumulated in PSUM (partition-axis reduction).
  - normalization: 1/rowsum is partition-broadcast on GpSimd and applied
    to mid^T = w^T-weighted numerator with one Vector multiply; the
    output is stored TRANSPOSED [L, N] in DRAM and un-transposed on the
    host during the gather.
  - The attention*w matmul of block k-1 is software-pipelined against
    the QK/exp of block k so the Scalar engine's exp stream stays hidden.
"""

import os
import sys

import numpy as np

for _p in ("/opt/trn_rl_repo",):
    if _p not in sys.path and os.path.isdir(_p):
        sys.path.insert(0, _p)

B, N, L, H = 8, 4096, 256, 128
SCALER = 256.0
NB = 1024           # query-block (free dim of score tiles)
NBH = 512           # half block (one PSUM bank of fp32)
NT = N // NB        # 4 query blocks
MT = N // 128       # 32 key tiles of 128
P = 128


def _build():
    import concourse.bass as bass
    import concourse.tile as tile
    from concourse import bacc, bass_isa, mybir
    from contextlib import ExitStack

    f32 = mybir.dt.float32
    f16 = mybir.dt.float16
    bf16 = mybir.dt.bfloat16
    AF = mybir.ActivationFunctionType

    nc = bacc.Bacc(
        "TRN2", target_bir_lowering=False, debug=False, num_devices=B
    )

    xT_ext = nc.declare_dram_parameter("xT", [L, N], f16, isOutput=False)
    wq_ext = nc.declare_dram_parameter("Wq", [L, H], f16, isOutput=False)
    wk_ext = nc.declare_dram_parameter("Wk", [L, H], f16, isOutput=False)
    vd_ext = nc.declare_dram_parameter("Vd", [L, H], f16, isOutput=False)
    vu_ext = nc.declare_dram_parameter("Vu", [H, L], f16, isOutput=False)
    # output stored transposed [L, N]; host un-transposes at gather
    out_ext = nc.declare_dram_parameter("out", [L, N], f16, isOutput=True)

    with tile.TileContext(nc) as tc, ExitStack() as ctx:
        persist = ctx.enter_context(tc.tile_pool(name="persist", bufs=1))

        ones_bf = persist.tile([P, 1], bf16)
        nc.gpsimd.memset(ones_bf[:], 1.0)
        # touch Exp right away so the ~2.7us ACT table load overlaps the
        # input DMAs instead of delaying the first real exp
        dum = persist.tile([1, 2], f32)
        nc.gpsimd.memset(dum[:], 0.0)
        nc.scalar.activation(dum[:, 1:2], dum[:, 0:1], AF.Exp)
        wrm = persist.tile([P, NBH], bf16, name="wrm")
        nc.vector.memset(wrm[:], 0.0)

        qw16 = persist.tile([P, 2 * H], f16)    # Q   [l_chunk][l_in, h]
        kw16 = persist.tile([P, 2 * H], f16)
        vd16 = persist.tile([P, 2 * H], f16)    # V_down [l_chunk][l_in, h]
        vu16 = persist.tile([P, L], f16)        # V_up   [h, l]
        vu_bf = persist.tile([P, L], bf16)      # V_up as bf16 (out matmul)
        xt16 = [
            [
                persist.tile(
                    [P, 1024], f16, tag=f"xt{c}_{s}", name=f"xt16_{c}_{s}"
                )
                for s in range(4)
            ]
            for c in range(2)
        ]
        qT16 = persist.tile([P, N], f16)        # q.T       [h, n]
        kT16 = persist.tile([P, N], f16)        # k.T       [h, m]
        w_sb = persist.tile([P, MT * H], bf16)  # x@V_down  [m_tile][m_in, h]

        # ---------------- phase A: direct fp16 loads ----------------
        # the first x chunk (s=0) is the critical path for the first QK
        # tiles - issue it before anything else on the serial issue path
        def dma_xt(s):
            for c in range(2):
                for h2 in range(2):
                    nc.sync.dma_start(
                        xt16[c][s][:, h2 * 512:(h2 + 1) * 512],
                        xT_ext[
                            c * P:(c + 1) * P,
                            s * 1024 + h2 * 512: s * 1024 + (h2 + 1) * 512,
                        ],
                    )
        dma_xt(0)
        for c in range(2):
            nc.sync.dma_start(qw16[:, c * H:(c + 1) * H], wq_ext[c * P:(c + 1) * P, :])
            nc.sync.dma_start(kw16[:, c * H:(c + 1) * H], wk_ext[c * P:(c + 1) * P, :])
            nc.sync.dma_start(vd16[:, c * H:(c + 1) * H], vd_ext[c * P:(c + 1) * P, :])
        nc.sync.dma_start(vu16[:], vu_ext[:, :])
        nc.vector.tensor_copy(vu_bf[:], vu16[:])
        for s in range(1, 4):
            dma_xt(s)

        # ------------- phases B+C: projections fused with attention -------
        with (
            tc.tile_pool(name="est", bufs=40) as est_pool,
            tc.tile_pool(name="sb_small", bufs=4) as sb_small,
            tc.tile_pool(name="outfin", bufs=4) as outfin_pool,
            tc.tile_pool(name="stp", bufs=3, space="PSUM") as stp,
            tc.tile_pool(name="mtp", bufs=1, space="PSUM") as mtp,
        ):
            est = {}      # (k, mt) -> bf16 [128, NB] exp score tiles
            mtiles = {}   # k -> psum numerator mid^T [h, n] tile
            mscs = {}     # k -> normalized mid (f16, SBUF)
            bc = {}       # k -> [128, NB] f32 broadcast 1/rowsum
            tree = {}     # (k, level, i) -> partial rowsum tiles

            def proj_qkT(w16, dst, f):
                ps = stp.tile([P, NB], f32, tag="stp", name=f"pjp_{f}")
                for c in range(2):
                    nc.tensor.matmul(
                        ps[:, :NBH],
                        w16[:, c * H:(c + 1) * H],
                        xt16[c][f // 2][:, (f % 2) * NBH:(f % 2 + 1) * NBH],
                        start=(c == 0), stop=(c == 1),
                    )
                nc.vector.tensor_copy(dst[:, f * NBH:(f + 1) * NBH], ps[:, :NBH])

            def proj_w(mt):
                ps = stp.tile([P, NB], f32, tag="stp", name=f"pjw_{mt}")
                for c in range(2):
                    nc.tensor.matmul(
                        ps[:, :H],
                        xt16[c][mt // 8][:, (mt % 8) * P:(mt % 8 + 1) * P],
                        vd16[:, c * H:(c + 1) * H],
                        start=(c == 0), stop=(c == 1),
                    )
                nc.vector.tensor_copy(w_sb[:, mt * H:(mt + 1) * H], ps[:, :H])

            def qk_exp(k, mt):
                ps = stp.tile([P, NB], f32, tag="stp", name=f"qk_{k}_{mt}")
                for h in range(2):
                    nc.tensor.matmul(
                        ps[:, h * NBH:(h + 1) * NBH],
                        kT16[:, mt * P:(mt + 1) * P],
                        qT16[:, k * NB + h * NBH: k * NB + (h + 1) * NBH],
                        start=True, stop=True,
                    )
                e = est_pool.tile([P, NB], bf16, tag="est", name=f"est_{k}_{mt}")
                est[(k, mt)] = e
                nc.scalar.activation(e[:], ps[:], AF.Exp, scale=1.0 / SCALER)

            def tree_adds(k, mt):
                # lazily build the 5-level pairwise rowsum tree on DVE;
                # levels 4/5 accumulate in fp32
                if mt % 2 == 1:
                    t = sb_small.tile([P, NB], bf16, tag="t1", bufs=3,
                                      name=f"t1_{k}_{mt}")
                    nc.vector.tensor_add(t[:], est[(k, mt - 1)][:], est[(k, mt)][:])
                    tree[(k, 1, mt // 2)] = t
                if mt % 4 == 3:
                    t = sb_small.tile([P, NB], bf16, tag="t2", bufs=3,
                                      name=f"t2_{k}_{mt}")
                    nc.vector.tensor_add(
                        t[:], tree[(k, 1, mt // 2 - 1)][:], tree[(k, 1, mt // 2)][:]
                    )
                    tree[(k, 2, mt // 4)] = t
                if mt % 8 == 7:
                    t = sb_small.tile([P, NB], bf16, tag="t3", bufs=3,
                                      name=f"t3_{k}_{mt}")
                    nc.vector.tensor_add(
                        t[:], tree[(k, 2, mt // 4 - 1)][:], tree[(k, 2, mt // 4)][:]
                    )
                    tree[(k, 3, mt // 8)] = t
                if mt % 16 == 15:
                    t = sb_small.tile([P, NB], f32, tag="t4", bufs=2,
                                      name=f"t4_{k}_{mt}")
                    nc.vector.tensor_add(
                        t[:], tree[(k, 3, mt // 8 - 1)][:], tree[(k, 3, mt // 8)][:]
                    )
                    tree[(k, 4, mt // 16)] = t
                if mt == 31:
                    t = sb_small.tile([P, NB], f32, tag="t5", bufs=2,
                                      name=f"t5_{k}")
                    nc.vector.tensor_add(
                        t[:], tree[(k, 4, 0)][:], tree[(k, 4, 1)][:]
                    )
                    tree[(k, 5, 0)] = t

            def rowsum_finish(k):
                # all-reduce over partitions on GpSimd (systolic daisy chain,
                # broadcast result), then fast reciprocal on DVE
                rsb = sb_small.tile([P, NB], f32, tag="rsb", bufs=2,
                                    name=f"rsb_{k}")
                nc.gpsimd.partition_all_reduce(
                    rsb[:], tree[(k, 5, 0)][:], channels=P,
                    reduce_op=bass_isa.ReduceOp.add,
                )
                bc[k] = rsb

            def recip_bc(k):
                # deferred so the Vector FIFO never head-of-line blocks on
                # the GpSimd PartitionAllReduce (finished a while ago)
                bck = sb_small.tile([P, NB], f32, tag="bc", bufs=2,
                                    name=f"bc_{k}")
                nc.vector.reciprocal_approx_fast(bck[:], bc[k][:])
                bc[k] = bck

            def norm_mid(k):
                # plain copy on the Scalar engine: it rides the exp stream,
                # so the mid PSUM tile frees right on schedule and the next
                # block's first PV matmul never stalls. The 1/rowsum scale
                # moves to the fin stage (linear, commutes with V_up).
                msc = sb_small.tile([P, NB], bf16, tag="msc", bufs=2,
                                    name=f"msc_{k}")
                nc.scalar.activation(msc[:], mtiles[k][:], AF.Copy)
                mscs[k] = msc

            def drain_out(k):
                # apply V_up, normalize by 1/rowsum, store transposed (f16)
                for lt in range(2):
                    op = stp.tile([P, NB], f32, tag="stp", name=f"op_{k}_{lt}")
                    for h in range(2):
                        nc.tensor.matmul(
                            op[:, h * NBH:(h + 1) * NBH],
                            vu_bf[:, lt * P:(lt + 1) * P],
                            mscs[k][:, h * NBH:(h + 1) * NBH],
                            start=True, stop=True,
                        )
                    fin = outfin_pool.tile([P, NB], f16, tag="fin")
                    nc.vector.tensor_mul(fin[:], op[:], bc[k][:])
                    nc.gpsimd.dma_start(
                        out_ext[lt * P:(lt + 1) * P, k * NB:(k + 1) * NB],
                        fin[:],
                    )

            def pv2(kk, j, mid):
                for h in range(2):
                    nc.tensor.matmul(
                        mid[:, h * NBH:(h + 1) * NBH],
                        w_sb[:, j * H:(j + 1) * H],
                        est[(kk, j)][:, h * NBH:(h + 1) * NBH],
                        start=(j == 0), stop=(j == MT - 1),
                    )

            # PE warm-up: junk matmuls while the x DMA is in flight, so the
            # HAM clock gate is already at 2.4 GHz when real work starts
            for i in range(16):
                ps = stp.tile([P, NB], f32, tag="stp", name=f"warm_{i}")
                nc.tensor.matmul(
                    ps[:, :NBH], wrm[:, :P], wrm[:], start=True, stop=True
                )

            # Uniform half-block-lagged schedule: during block k, PE runs
            # QK(k) plus the oldest pending attention@w work (last half of
            # block k-1, then first half of block k), so per-mt PE load is a
            # constant 4 matmuls and the Scalar engine's exp stream paces
            # everything. Block 0 uses the projection matmuls as its filler.
            # head: the first QK tiles need qT/kT half-blocks 0,1 (chunk s0)
            proj_qkT(qw16, qT16, 0)
            proj_qkT(qw16, qT16, 1)
            proj_qkT(kw16, kT16, 0)
            proj_qkT(kw16, kT16, 1)

            for k in range(NT):
                for mt in range(MT):
                    qk_exp(k, mt)
                    if k == 0:
                        # one w-projection per mt keeps w 16 tiles ahead of
                        # its consumer PV(0, mt-16)
                        proj_w(mt)
                        if mt % 4 == 0 and mt < 24:
                            proj_qkT(kw16, kT16, mt // 4 + 2)
                        if mt in (8, 10):
                            proj_qkT(qw16, qT16, (mt - 8) // 2 + 2)
                    if k == 1 and mt in (5, 9, 13, 17):
                        proj_qkT(qw16, qT16, (mt - 5) // 4 + 4)
                    if k >= 1 and mt <= 15:
                        pv2(k - 1, 16 + mt, mtiles[k - 1])
                    if mt == 16:
                        mid = mtp.tile([P, NB], f32, tag="mtp", name=f"mid_{k}")
                        mtiles[k] = mid
                    if mt >= 16:
                        pv2(k, mt - 16, mtiles[k])
                    if k >= 1 and mt == 10:
                        recip_bc(k - 1)
                    if k >= 1 and mt == 15:
                        norm_mid(k - 1)
                    tree_adds(k, mt)
                    if k >= 1 and mt == 18:
                        drain_out(k - 1)
                rowsum_finish(k)

            # epilogue: finish block 3's product and drain it
            recip_bc(NT - 1)
            k3 = NT - 1
            for h in range(2):
                for j in range(16, MT):
                    nc.tensor.matmul(
                        mtiles[k3][:, h * NBH:(h + 1) * NBH],
                        w_sb[:, j * H:(j + 1) * H],
                        est[(k3, j)][:, h * NBH:(h + 1) * NBH],
                        start=(j == 0), stop=(j == MT - 1),
                    )
                # drain this half as soon as its chain closes
                msc_h = sb_small.tile([P, NBH], bf16, tag="msch", bufs=2,
                                      name=f"msch_{h}")
                nc.scalar.activation(
                    msc_h[:], mtiles[k3][:, h * NBH:(h + 1) * NBH], AF.Copy
                )
                for lt in range(2):
                    op = stp.tile([P, NB], f32, tag="stp", name=f"ope_{h}_{lt}")
                    nc.tensor.matmul(
                        op[:, :NBH], vu_bf[:, lt * P:(lt + 1) * P], msc_h[:],
                        start=True, stop=True,
                    )
                    fin = outfin_pool.tile([P, NBH], f16, tag="fine", bufs=4)
                    nc.vector.tensor_mul(
                        fin[:], op[:, :NBH],
                        bc[k3][:, h * NBH:(h + 1) * NBH],
                    )
                    nc.gpsimd.dma_start(
                        out_ext[
                            lt * P:(lt + 1) * P,
                            k3 * NB + h * NBH: k3 * NB + (h + 1) * NBH,
                        ],
                        fin[:],
                    )

    if not nc.is_finalized():
        nc.finalize()
    return nc


_GRAPH_CACHE = {}


def _get_graph():
    if "nc" not in _GRAPH_CACHE:
        _GRAPH_CACHE["nc"] = _build()
    return _GRAPH_CACHE["nc"]


def run(inputs: dict, trace: bool = False):
    """Run the SPMD kernel on 8 cores. Returns (output, BassKernelResults)."""
    from concourse.bass_utils import run_bass_kernel_spmd

    x = np.asarray(inputs["x"], dtype=np.float32)
    Q = np.asarray(inputs["Q"], dtype=np.float32)[0]
    K = np.asarray(inputs["K"], dtype=np.float32)[0]
    Vd = np.asarray(inputs["V_down"], dtype=np.float32)[0]
    Vu = np.asarray(inputs["V_up"], dtype=np.float32)[0]

    wq = np.ascontiguousarray(Q).astype(np.float16)
    wk = np.ascontiguousarray(K).astype(np.float16)
    vd = np.ascontiguousarray(Vd).astype(np.float16)
    vu = np.ascontiguousarray(Vu).astype(np.float16)

    in_maps = []
    for b in range(B):
        in_maps.append({
            "xT": np.ascontiguousarray(x[b].T).astype(np.float16),
            "Wq": wq,
            "Wk": wk,
            "Vd": vd,
            "Vu": vu,
        })

    nc = _get_graph()
    res = run_bass_kernel_spmd(nc, in_maps, core_ids=list(range(B)), trace=trace)
    # device output is [L, N] per core; un-transpose during the gather
    out = np.stack([np.asarray(res.results[i]["out"]).astype(np.float32).T for i in range(B)])
    return np.ascontiguousarray(out, dtype=np.float32), res


def kernel(**inputs) -> np.ndarray:
    out, _ = run(inputs, trace=False)
    return out
